# revision 5
# baseline (speedup 1.0000x reference)
"""Trainium2 Bass kernel for nn_AttenPool_22917945491863.

Mathematical reduction: in the reference, ``attn`` is softmaxed over axis 3
and then summed over that same axis — the sum of a softmax over its own axis
is exactly 1, so the whole query branch (2 convs, BN, ReLU, LayerNorm,
softmax) collapses to ``a = ones``. The remaining computation

    out = sumpool4x4((1-alpha) * (conv3x3(bn(x), wv) + bv) + alpha * x)

is a 6x6 stride-4 convolution over zero-padded x (sumpool of a 3x3 conv is a
6x6 stride-4 conv with summed taps; the BN scale folds into the weights; the
BN shift and conv bias fold into a precomputed per-output-position bias map;
the alpha*x sum-pool folds in as a depthwise component on the central 4x4
taps).

Device mapping (8 cores, batch-parallel, 2 samples each):
  - x is pre-shuffled on the host into a zero-padded h-parity layout
    [128, 65*130]: partition p holds channel (p % 64); partitions 0-63 hold
    even padded rows, 64-127 odd padded rows. Each matmul then contracts
    over K=128 = 64 channels x 2 vertically-adjacent taps.
  - The 36 conv taps become 18 tap-pair matmuls [K=128, M=64, N=512]
    (dtype float32r: fp32 storage, ~14-bit-mantissa multiply at bf16 speed)
    accumulated in PSUM, two N=512 output tiles per sample.
  - Epilogue: PSUM + bias map on DVE, DMA out.
"""

import numpy as np

B, C, H, W = 16, 64, 128, 128
NCORES = 8
BPC = B // NCORES  # samples per core
OH = OW = 32  # output spatial
WPAD = 130  # padded row length (zero col at 0 and 129)
NROW = 65  # padded rows per parity block
EPS = 1e-5

MM_DTYPE = "f32r"  # "f32r" | "bf16"

_PROGRAM_CACHE = {}


def _build_program():
    import concourse.bacc as bacc
    import concourse.mybir as mybir
    import concourse.tile as tile

    f32 = mybir.dt.float32
    if MM_DTYPE == "bf16":
        x_dt = w_dt = mybir.dt.bfloat16
    else:
        # fp32r: fp32 storage, reduced-mantissa matmul at bf16 speed
        # (N>=256). The whole producer chain must be declared float32r for
        # the BIR verifier; HW accepts unrounded fp32 bits (measured
        # rel err 1.8e-4 on [128,128]x[128,512] vs f64).
        x_dt = w_dt = mybir.dt.float32r

    nc = bacc.Bacc("TRN2", target_bir_lowering=False, debug=False,
                   num_devices=NCORES)
    # host-padded parity-layout x: [b, 128, NROW*WPAD]
    xp = nc.dram_tensor("xp", [BPC, 128, NROW * WPAD], x_dt,
                        kind="ExternalInput").ap()
    # lhsT tap-pair weights, k-major: [128, 18*64]
    w_in = nc.dram_tensor("w", [128, 18 * 64], w_dt, kind="ExternalInput").ap()
    # bias map [64, 1024] f32
    ab_in = nc.dram_tensor("abias", [C, OH * OW], f32, kind="ExternalInput").ap()
    out = nc.dram_tensor("out", [BPC, C, OH * OW], f32, kind="ExternalOutput").ap()

    dma_x = nc.gpsimd if MM_DTYPE == "bf16" else nc.sync

    with tile.TileContext(nc) as tc:
        with (
            tc.tile_pool(name="consts", bufs=1) as consts,
            tc.tile_pool(name="xpool", bufs=2) as xpool,
            tc.tile_pool(name="opool", bufs=4) as opool,
            tc.tile_pool(name="pspool", bufs=4, space="PSUM") as pspool,
        ):
            w_sb = consts.tile([128, 18 * 64], w_dt)
            nc.sync.dma_start(out=w_sb[:], in_=w_in[:])
            ab_sb = consts.tile([C, OH * OW], f32)
            nc.sync.dma_start(out=ab_sb[:], in_=ab_in[:])

            for b in range(BPC):
                x2 = xpool.tile([128, NROW * WPAD], x_dt)
                # two row-chunks so compute on the first half can start
                # while the second half is still loading
                for r0, r1 in ((0, 33), (33, NROW)):
                    dma_x.dma_start(
                        out=x2[:, r0 * WPAD:r1 * WPAD],
                        in_=xp[b, :, r0 * WPAD:r1 * WPAD],
                    )
                v = x2[:].rearrange("p (r c) -> p r c", c=WPAD)
                for half in range(2):
                    ps = pspool.tile([C, 512], f32)
                    for i in range(18):
                        a, sw = divmod(i, 6)
                        r0 = 32 * half + a
                        rhs = v[:, r0: r0 + 31: 2, sw: sw + 125: 4]
                        lhsT = w_sb[:, i * 64:(i + 1) * 64]
                        nc.tensor.matmul(ps[:], lhsT, rhs,
                                         start=(i == 0), stop=(i == 17))
                    ob = opool.tile([C, 512], f32)
                    nc.vector.tensor_add(
                        ob[:], ps[:], ab_sb[:, half * 512:(half + 1) * 512])
                    nc.sync.dma_start(
                        out=out[b, :, half * 512:(half + 1) * 512], in_=ob[:])

    nc.compile()
    return nc


def _host_precompute(inputs):
    """Fold BN/alpha/bias into 6x6 stride-4 conv weights + bias map (f64)."""
    g0 = np.asarray(inputs["g0"], np.float64)
    b0 = np.asarray(inputs["b0"], np.float64)
    m0 = np.asarray(inputs["m0"], np.float64)
    v0 = np.asarray(inputs["v0"], np.float64)
    wv = np.asarray(inputs["wv"], np.float64)
    bv = np.asarray(inputs["bv"], np.float64)
    alpha = float(np.asarray(inputs["alpha"]))

    s0 = g0 / np.sqrt(v0 + EPS)
    t0 = b0 - m0 * s0

    # W'[o,c,sh,sw] = sum of 3x3 taps t with s - t in [0,4)^2
    Wp = np.zeros((C, C, 6, 6))
    for sh in range(6):
        for sw in range(6):
            th0, th1 = max(0, sh - 3), min(3, sh + 1)
            tw0, tw1 = max(0, sw - 3), min(3, sw + 1)
            Wp[:, :, sh, sw] = wv[:, :, th0:th1, tw0:tw1].sum(axis=(2, 3))

    W_final = (1.0 - alpha) * Wp * s0[None, :, None, None]
    idx = np.arange(C)
    for sh in range(1, 5):
        for sw in range(1, 5):
            W_final[idx, idx, sh, sw] += alpha

    # bias map: contribution of the BN shift t0 through the conv (with
    # zero-padding mask) plus conv bias, scaled by (1-alpha)
    Rm = np.zeros((OH, 6))
    for p in range(OH):
        for s in range(6):
            if 0 <= 4 * p + s - 1 < H:
                Rm[p, s] = 1.0
    A0 = np.einsum("ocuv,pu,qv,c->opq", Wp, Rm, Rm, t0)
    Abias = (1.0 - alpha) * (A0 + 16.0 * bv[:, None, None])

    # lhsT tap-pair layout: pair i = (a, sw), rows 0-63 = tap (2a, sw),
    # rows 64-127 = tap (2a+1, sw); [k, i*64 + m] with k=ci, m=co
    W18 = np.zeros((128, 18 * 64))
    for i in range(18):
        a, sw = divmod(i, 6)
        W18[0:64, i * 64:(i + 1) * 64] = W_final[:, :, 2 * a, sw].T
        W18[64:128, i * 64:(i + 1) * 64] = W_final[:, :, 2 * a + 1, sw].T

    return W18, Abias.reshape(C, OH * OW)


def _host_shuffle_x(x):
    """Zero-padded h-parity layout: [B, 128, NROW, WPAD] f32.

    Partition p < 64: channel p, even padded rows (pad row 2*r -> h=2r-1);
    partition p >= 64: channel p-64, odd padded rows (pad row 2*r+1 -> h=2r).
    Data cols 1..128; col 0 and col 129 are the zero padding.
    """
    xpad = np.zeros((B, 128, NROW, WPAD), np.float32)
    xpad[:, 0:64, 1:65, 1:129] = x[:, :, 1::2, :]
    xpad[:, 64:128, 0:64, 1:129] = x[:, :, 0::2, :]
    return xpad.reshape(B, 128, NROW * WPAD)


def kernel(**inputs):
    from concourse.bass_utils import run_bass_kernel_spmd

    x = np.asarray(inputs["x"], np.float32)
    W18, Abias = _host_precompute(inputs)

    if MM_DTYPE == "bf16":
        import ml_dtypes
        w_host = W18.astype(ml_dtypes.bfloat16)
    else:
        w_host = W18.astype(np.float32)
    ab_host = Abias.astype(np.float32)
    xp = _host_shuffle_x(x)

    if "nc" not in _PROGRAM_CACHE:
        _PROGRAM_CACHE["nc"] = _build_program()
    nc = _PROGRAM_CACHE["nc"]

    in_maps = [
        {"xp": xp[i * BPC:(i + 1) * BPC], "w": w_host, "abias": ab_host}
        for i in range(NCORES)
    ]
    res = run_bass_kernel_spmd(nc, in_maps, list(range(NCORES)))
    out = np.concatenate(
        [res.results[i]["out"].reshape(BPC, C, OH, OW) for i in range(NCORES)],
        axis=0,
    )
    return np.ascontiguousarray(out.astype(np.float32))


# revision 9
# speedup vs baseline: 1.3230x; 1.3230x over previous
"""Trainium2 Bass kernel for nn_AttenPool_22917945491863.

Mathematical reduction: in the reference, ``attn`` is softmaxed over axis 3
and then summed over that same axis — the sum of a softmax over its own axis
is exactly 1, so the whole query branch (2 convs, BN, ReLU, LayerNorm,
softmax) collapses to ``a = ones``. The remaining computation

    out = sumpool4x4((1-alpha) * (conv3x3(bn(x), wv) + bv) + alpha * x)

is a 6x6 stride-4 convolution over zero-padded x (sumpool of a 3x3 conv is a
6x6 stride-4 conv with summed taps; the BN scale folds into the weights; the
BN shift and conv bias fold into a precomputed per-output-position bias map;
the alpha*x sum-pool folds in as a depthwise component on the central 4x4
taps).

Device mapping (8 cores, batch-parallel, 2 samples each):
  - x is pre-shuffled on the host into a zero-padded h-parity, phase-major
    column layout [128, 65*132]: partition p holds channel (p % 64);
    partitions 0-63 hold even padded rows, 64-127 odd padded rows; padded
    col c sits at (c%4)*33 + c//4 within a row so each tap's 32 stride-4
    columns are contiguous in SBUF. Each matmul contracts over K=128 =
    64 channels x 2 vertically-adjacent taps.
  - The 36 conv taps become 18 tap-pair matmuls [K=128, M=64, N=512]
    (dtype float32r: fp32 storage, reduced-mantissa multiply at ~2x fp32
    speed) accumulated in PSUM; two N=512 output tiles per sample.
  - Raw engine blocks with manual semaphores (no Tile framework): Sync
    streams the x chunks, ACT loads weights/bias and drains outputs,
    PE runs the 72 matmuls, DVE adds the bias map from PSUM.
"""

import numpy as np

B, C, H, W = 16, 64, 128, 128
NCORES = 8
BPC = B // NCORES  # samples per core
OH = OW = 32  # output spatial
WPAD = 132  # padded row length: stored phase-major as [4 phases][33 cols]
NROW = 65  # padded rows per parity block
EPS = 1e-5
NT = 2 * BPC  # output tiles (sample x half)

_PROGRAM_CACHE = {}


def _build_program():
    import concourse.bacc as bacc
    import concourse.mybir as mybir

    f32 = mybir.dt.float32
    # fp32r: fp32 storage, reduced-mantissa matmul (measured rel err 1.8e-4
    # on [128,128]x[128,512] vs f64). The whole producer chain must be
    # declared float32r for the BIR verifier; HW accepts unrounded fp32.
    xdt = mybir.dt.float32r

    nc = bacc.Bacc("TRN2", target_bir_lowering=False, debug=False,
                   num_devices=NCORES)
    xp = nc.dram_tensor("xp", [BPC, 128, NROW * WPAD], xdt,
                        kind="ExternalInput").ap()
    w_in = nc.dram_tensor("w", [128, 18 * 64], xdt, kind="ExternalInput").ap()
    ab_in = nc.dram_tensor("abias", [C, OH * OW], f32,
                           kind="ExternalInput").ap()
    out = nc.dram_tensor("out", [BPC, C, OH * OW], f32,
                         kind="ExternalOutput").ap()

    x2 = [nc.alloc_sbuf_tensor(f"x2_{b}", [128, NROW * WPAD], xdt).ap()
          for b in range(BPC)]
    w_sb = nc.alloc_sbuf_tensor("w_sb", [128, 18 * 64], xdt).ap()
    ab_sb = nc.alloc_sbuf_tensor("ab_sb", [C, OH * OW], f32).ap()
    ob = [nc.alloc_sbuf_tensor(f"ob_{t}", [C, 512], f32).ap()
          for t in range(NT)]
    ps = [nc.alloc_psum_tensor(f"ps_{t}", [C, 512], f32).ap()
          for t in range(NT)]

    dsem = nc.alloc_semaphore("dsem")   # x chunk DMAs (sync ring)
    wsem = nc.alloc_semaphore("wsem")   # w + abias DMAs (scalar ring)
    mmsem = nc.alloc_semaphore("mmsem")  # per-tile matmul group done
    vsem = nc.alloc_semaphore("vsem")   # per-tile bias add done
    osem = nc.alloc_semaphore("osem")   # output DMAs landed

    # x chunk row ranges; tile (b, half) consumes chunks up to index
    # 2*b + half + 1 (half 0 needs rows 0..32, half 1 rows 32..64)
    CHUNKS = [(0, 33), (33, NROW)]

    with nc.Block() as block:

        @block.sync
        def _(sync):
            for b in range(BPC):
                for r0, r1 in CHUNKS:
                    sync.dma_start(
                        out=x2[b][:, r0 * WPAD:r1 * WPAD],
                        in_=xp[b, :, r0 * WPAD:r1 * WPAD],
                    ).then_inc(dsem, 16)

        @block.scalar
        def _(scalar):
            scalar.dma_start(out=w_sb[:], in_=w_in[:]).then_inc(wsem, 16)
            scalar.dma_start(out=ab_sb[:], in_=ab_in[:]).then_inc(wsem, 16)
            for t in range(NT):
                b, half = divmod(t, 2)
                scalar.wait_ge(vsem, t + 1)
                scalar.dma_start(
                    out=out[b, :, half * 512:(half + 1) * 512],
                    in_=ob[t][:],
                ).then_inc(osem, 16)
            scalar.wait_ge(osem, 16 * NT)

        @block.tensor
        def _(tensor):
            tensor.wait_ge(wsem, 16)
            for t in range(NT):
                b, half = divmod(t, 2)
                tensor.wait_ge(dsem, 16 * (t + 1))
                v = x2[b].rearrange("p (r f c) -> p r f c", f=4, c=33)
                for i in range(18):
                    a, sw = divmod(i, 6)
                    r0 = 32 * half + a
                    rhs = v[:, r0: r0 + 31: 2, sw % 4, sw // 4: sw // 4 + 32]
                    mm = tensor.matmul(ps[t][:], w_sb[:, i * 64:(i + 1) * 64],
                                       rhs, start=(i == 0), stop=(i == 17))
                    if i == 17:
                        mm.then_inc(mmsem, 1)

        @block.vector
        def _(vector):
            vector.wait_ge(wsem, 32)
            for t in range(NT):
                b, half = divmod(t, 2)
                vector.wait_ge(mmsem, t + 1)
                vector.tensor_add(
                    ob[t][:], ps[t][:],
                    ab_sb[:, half * 512:(half + 1) * 512],
                ).then_inc(vsem, 1)

    nc.compile()
    return nc


def _host_precompute(inputs):
    """Fold BN/alpha/bias into 6x6 stride-4 conv weights + bias map (f64)."""
    g0 = np.asarray(inputs["g0"], np.float64)
    b0 = np.asarray(inputs["b0"], np.float64)
    m0 = np.asarray(inputs["m0"], np.float64)
    v0 = np.asarray(inputs["v0"], np.float64)
    wv = np.asarray(inputs["wv"], np.float64)
    bv = np.asarray(inputs["bv"], np.float64)
    alpha = float(np.asarray(inputs["alpha"]))

    s0 = g0 / np.sqrt(v0 + EPS)
    t0 = b0 - m0 * s0

    # W'[o,c,sh,sw] = sum of 3x3 taps t with s - t in [0,4)^2
    Wp = np.zeros((C, C, 6, 6))
    for sh in range(6):
        for sw in range(6):
            th0, th1 = max(0, sh - 3), min(3, sh + 1)
            tw0, tw1 = max(0, sw - 3), min(3, sw + 1)
            Wp[:, :, sh, sw] = wv[:, :, th0:th1, tw0:tw1].sum(axis=(2, 3))

    W_final = (1.0 - alpha) * Wp * s0[None, :, None, None]
    idx = np.arange(C)
    for sh in range(1, 5):
        for sw in range(1, 5):
            W_final[idx, idx, sh, sw] += alpha

    # bias map: contribution of the BN shift t0 through the conv (with
    # zero-padding mask) plus conv bias, scaled by (1-alpha)
    Rm = np.zeros((OH, 6))
    for p in range(OH):
        for s in range(6):
            if 0 <= 4 * p + s - 1 < H:
                Rm[p, s] = 1.0
    A0 = np.einsum("ocuv,pu,qv,c->opq", Wp, Rm, Rm, t0)
    Abias = (1.0 - alpha) * (A0 + 16.0 * bv[:, None, None])

    # lhsT tap-pair layout: pair i = (a, sw), rows 0-63 = tap (2a, sw),
    # rows 64-127 = tap (2a+1, sw); [k, i*64 + m] with k=ci, m=co
    W18 = np.zeros((128, 18 * 64))
    for i in range(18):
        a, sw = divmod(i, 6)
        W18[0:64, i * 64:(i + 1) * 64] = W_final[:, :, 2 * a, sw].T
        W18[64:128, i * 64:(i + 1) * 64] = W_final[:, :, 2 * a + 1, sw].T

    return W18, Abias.reshape(C, OH * OW)


def _host_shuffle_x(x):
    """Zero-padded h-parity, phase-major-column layout [B, 128, NROW*WPAD].

    Partition p < 64: channel p, even padded rows (pad row 2*r -> h=2r-1);
    partition p >= 64: channel p-64, odd padded rows (pad row 2*r+1 -> h=2r).
    Padded col c (data cols 1..128, zeros at 0/129/130/131) is stored at
    row offset (c%4)*33 + c//4 so stride-4 tap reads are contiguous.
    """
    xpad = np.zeros((B, 128, NROW, WPAD), np.float32)
    xpad[:, 0:64, 1:65, 1:129] = x[:, :, 1::2, :]
    xpad[:, 64:128, 0:64, 1:129] = x[:, :, 0::2, :]
    # c = cc*4 + phase -> phase-major [4][33]
    xph = xpad.reshape(B, 128, NROW, 33, 4).transpose(0, 1, 2, 4, 3)
    return np.ascontiguousarray(xph).reshape(B, 128, NROW * WPAD)


def kernel(**inputs):
    from concourse.bass_utils import run_bass_kernel_spmd

    x = np.asarray(inputs["x"], np.float32)
    W18, Abias = _host_precompute(inputs)
    w_host = W18.astype(np.float32)
    ab_host = Abias.astype(np.float32)
    xp = _host_shuffle_x(x)

    if "nc" not in _PROGRAM_CACHE:
        _PROGRAM_CACHE["nc"] = _build_program()
    nc = _PROGRAM_CACHE["nc"]

    in_maps = [
        {"xp": xp[i * BPC:(i + 1) * BPC], "w": w_host, "abias": ab_host}
        for i in range(NCORES)
    ]
    res = run_bass_kernel_spmd(nc, in_maps, list(range(NCORES)))
    out = np.concatenate(
        [res.results[i]["out"].reshape(BPC, C, OH, OW) for i in range(NCORES)],
        axis=0,
    )
    return np.ascontiguousarray(out.astype(np.float32))


# revision 10
# speedup vs baseline: 1.3822x; 1.0447x over previous
"""Trainium2 Bass kernel for nn_AttenPool_22917945491863.

Mathematical reduction: in the reference, ``attn`` is softmaxed over axis 3
and then summed over that same axis — the sum of a softmax over its own axis
is exactly 1, so the whole query branch (2 convs, BN, ReLU, LayerNorm,
softmax) collapses to ``a = ones``. The remaining computation

    out = sumpool4x4((1-alpha) * (conv3x3(bn(x), wv) + bv) + alpha * x)

is a 6x6 stride-4 convolution over zero-padded x (sumpool of a 3x3 conv is a
6x6 stride-4 conv with summed taps; the BN scale folds into the weights; the
BN shift and conv bias fold into a precomputed per-output-position bias map;
the alpha*x sum-pool folds in as a depthwise component on the central 4x4
taps).

Device mapping (8 cores, batch-parallel, 2 samples each):
  - x is pre-shuffled on the host into a zero-padded h-parity, phase-major
    column layout [128, 65*132]: partition p holds channel (p % 64);
    partitions 0-63 hold even padded rows, 64-127 odd padded rows; padded
    col c sits at (c%4)*33 + c//4 within a row so each tap's 32 stride-4
    columns are contiguous in SBUF. Each matmul contracts over K=128 =
    64 channels x 2 vertically-adjacent taps.
  - The 36 conv taps become 18 tap-pair matmuls [K=128, M=64, N=512]
    (dtype float32r: fp32 storage, reduced-mantissa multiply at ~2x fp32
    speed) accumulated in PSUM; two N=512 output tiles per sample.
  - Raw engine blocks with manual semaphores (no Tile framework): Sync
    streams the x chunks, ACT loads weights/bias and drains outputs,
    PE runs the 72 matmuls, DVE adds the bias map from PSUM.
"""

import numpy as np

B, C, H, W = 16, 64, 128, 128
NCORES = 8
BPC = B // NCORES  # samples per core
OH = OW = 32  # output spatial
WPAD = 132  # padded row length: stored phase-major as [4 phases][33 cols]
NROW = 65  # padded rows per parity block
EPS = 1e-5
NT = 2 * BPC  # output tiles (sample x half)

_PROGRAM_CACHE = {}


def _build_program():
    import concourse.bacc as bacc
    import concourse.mybir as mybir

    f32 = mybir.dt.float32
    # fp32r: fp32 storage, reduced-mantissa matmul (measured rel err 1.8e-4
    # on [128,128]x[128,512] vs f64). The whole producer chain must be
    # declared float32r for the BIR verifier; HW accepts unrounded fp32.
    xdt = mybir.dt.float32r

    nc = bacc.Bacc("TRN2", target_bir_lowering=False, debug=False,
                   num_devices=NCORES)
    xp = nc.dram_tensor("xp", [BPC, 128, NROW * WPAD], xdt,
                        kind="ExternalInput").ap()
    w_in = nc.dram_tensor("w", [128, 18 * 64], xdt, kind="ExternalInput").ap()
    ab_in = nc.dram_tensor("abias", [C, OH * OW], f32,
                           kind="ExternalInput").ap()
    out = nc.dram_tensor("out", [BPC, C, OH * OW], f32,
                         kind="ExternalOutput").ap()

    x2 = [nc.alloc_sbuf_tensor(f"x2_{b}", [128, NROW * WPAD], xdt).ap()
          for b in range(BPC)]
    w_sb = nc.alloc_sbuf_tensor("w_sb", [128, 18 * 64], xdt).ap()
    ab_sb = nc.alloc_sbuf_tensor("ab_sb", [C, OH * OW], f32).ap()
    ob = [nc.alloc_sbuf_tensor(f"ob_{t}", [C, 512], f32).ap()
          for t in range(NT)]
    ps = [nc.alloc_psum_tensor(f"ps_{t}", [C, 512], f32).ap()
          for t in range(NT)]

    dsem = nc.alloc_semaphore("dsem")   # x chunk DMAs (sync ring)
    wsem = nc.alloc_semaphore("wsem")   # w + abias DMAs (scalar ring)
    mmsem = nc.alloc_semaphore("mmsem")  # per-tile matmul group done
    vsem = nc.alloc_semaphore("vsem")   # per-tile bias add done
    osem = nc.alloc_semaphore("osem")   # output DMAs landed

    # x chunk row ranges; tile (b, half) consumes chunks up to index
    # 2*b + half + 1 (half 0 needs rows 0..32, half 1 rows 32..64)
    CHUNKS = [(0, 33), (33, NROW)]

    with nc.Block(no_gpsimd_drain=True) as block:

        @block.sync
        def _(sync):
            # single HWDGE FIFO, ordered by consumption: weights/bias first
            # (small, gate the PE), then the x chunks, then the outputs
            sync.dma_start(out=w_sb[:], in_=w_in[:]).then_inc(wsem, 16)
            sync.dma_start(out=ab_sb[:], in_=ab_in[:]).then_inc(wsem, 16)
            for b in range(BPC):
                for r0, r1 in CHUNKS:
                    sync.dma_start(
                        out=x2[b][:, r0 * WPAD:r1 * WPAD],
                        in_=xp[b, :, r0 * WPAD:r1 * WPAD],
                    ).then_inc(dsem, 16)
            for t in range(NT):
                b, half = divmod(t, 2)
                sync.wait_ge(vsem, t + 1)
                sync.dma_start(
                    out=out[b, :, half * 512:(half + 1) * 512],
                    in_=ob[t][:],
                ).then_inc(osem, 16)
            sync.wait_ge(osem, 16 * NT)

        @block.tensor
        def _(tensor):
            tensor.wait_ge(wsem, 16)
            for t in range(NT):
                b, half = divmod(t, 2)
                tensor.wait_ge(dsem, 16 * (t + 1))
                v = x2[b].rearrange("p (r f c) -> p r f c", f=4, c=33)
                for i in range(18):
                    a, sw = divmod(i, 6)
                    r0 = 32 * half + a
                    rhs = v[:, r0: r0 + 31: 2, sw % 4, sw // 4: sw // 4 + 32]
                    mm = tensor.matmul(ps[t][:], w_sb[:, i * 64:(i + 1) * 64],
                                       rhs, start=(i == 0), stop=(i == 17))
                    if i == 17:
                        mm.then_inc(mmsem, 1)

        @block.vector
        def _(vector):
            vector.wait_ge(wsem, 32)
            for t in range(NT):
                b, half = divmod(t, 2)
                vector.wait_ge(mmsem, t + 1)
                vector.tensor_add(
                    ob[t][:], ps[t][:],
                    ab_sb[:, half * 512:(half + 1) * 512],
                ).then_inc(vsem, 1)

    nc.compile()
    return nc


def _host_precompute(inputs):
    """Fold BN/alpha/bias into 6x6 stride-4 conv weights + bias map (f64)."""
    g0 = np.asarray(inputs["g0"], np.float64)
    b0 = np.asarray(inputs["b0"], np.float64)
    m0 = np.asarray(inputs["m0"], np.float64)
    v0 = np.asarray(inputs["v0"], np.float64)
    wv = np.asarray(inputs["wv"], np.float64)
    bv = np.asarray(inputs["bv"], np.float64)
    alpha = float(np.asarray(inputs["alpha"]))

    s0 = g0 / np.sqrt(v0 + EPS)
    t0 = b0 - m0 * s0

    # W'[o,c,sh,sw] = sum of 3x3 taps t with s - t in [0,4)^2
    Wp = np.zeros((C, C, 6, 6))
    for sh in range(6):
        for sw in range(6):
            th0, th1 = max(0, sh - 3), min(3, sh + 1)
            tw0, tw1 = max(0, sw - 3), min(3, sw + 1)
            Wp[:, :, sh, sw] = wv[:, :, th0:th1, tw0:tw1].sum(axis=(2, 3))

    W_final = (1.0 - alpha) * Wp * s0[None, :, None, None]
    idx = np.arange(C)
    for sh in range(1, 5):
        for sw in range(1, 5):
            W_final[idx, idx, sh, sw] += alpha

    # bias map: contribution of the BN shift t0 through the conv (with
    # zero-padding mask) plus conv bias, scaled by (1-alpha)
    Rm = np.zeros((OH, 6))
    for p in range(OH):
        for s in range(6):
            if 0 <= 4 * p + s - 1 < H:
                Rm[p, s] = 1.0
    A0 = np.einsum("ocuv,pu,qv,c->opq", Wp, Rm, Rm, t0)
    Abias = (1.0 - alpha) * (A0 + 16.0 * bv[:, None, None])

    # lhsT tap-pair layout: pair i = (a, sw), rows 0-63 = tap (2a, sw),
    # rows 64-127 = tap (2a+1, sw); [k, i*64 + m] with k=ci, m=co
    W18 = np.zeros((128, 18 * 64))
    for i in range(18):
        a, sw = divmod(i, 6)
        W18[0:64, i * 64:(i + 1) * 64] = W_final[:, :, 2 * a, sw].T
        W18[64:128, i * 64:(i + 1) * 64] = W_final[:, :, 2 * a + 1, sw].T

    return W18, Abias.reshape(C, OH * OW)


def _host_shuffle_x(x):
    """Zero-padded h-parity, phase-major-column layout [B, 128, NROW*WPAD].

    Partition p < 64: channel p, even padded rows (pad row 2*r -> h=2r-1);
    partition p >= 64: channel p-64, odd padded rows (pad row 2*r+1 -> h=2r).
    Padded col c (data cols 1..128, zeros at 0/129/130/131) is stored at
    row offset (c%4)*33 + c//4 so stride-4 tap reads are contiguous.
    """
    xpad = np.zeros((B, 128, NROW, WPAD), np.float32)
    xpad[:, 0:64, 1:65, 1:129] = x[:, :, 1::2, :]
    xpad[:, 64:128, 0:64, 1:129] = x[:, :, 0::2, :]
    # c = cc*4 + phase -> phase-major [4][33]
    xph = xpad.reshape(B, 128, NROW, 33, 4).transpose(0, 1, 2, 4, 3)
    return np.ascontiguousarray(xph).reshape(B, 128, NROW * WPAD)


def kernel(**inputs):
    from concourse.bass_utils import run_bass_kernel_spmd

    x = np.asarray(inputs["x"], np.float32)
    W18, Abias = _host_precompute(inputs)
    w_host = W18.astype(np.float32)
    ab_host = Abias.astype(np.float32)
    xp = _host_shuffle_x(x)

    if "nc" not in _PROGRAM_CACHE:
        _PROGRAM_CACHE["nc"] = _build_program()
    nc = _PROGRAM_CACHE["nc"]

    in_maps = [
        {"xp": xp[i * BPC:(i + 1) * BPC], "w": w_host, "abias": ab_host}
        for i in range(NCORES)
    ]
    res = run_bass_kernel_spmd(nc, in_maps, list(range(NCORES)))
    out = np.concatenate(
        [res.results[i]["out"].reshape(BPC, C, OH, OW) for i in range(NCORES)],
        axis=0,
    )
    return np.ascontiguousarray(out.astype(np.float32))
